# revision 2
# baseline (speedup 1.0000x reference)
"""Trainium2 kernel for nn_ColorLoss (retrieval_knn).

Computes mean_{b,m} min_n ||pred[b,m] - gt[b,n]|| for B=4, M=N=8192, D=3.

Strategy (8 NeuronCores, SPMD):
  - Shard queries over (batch, half-of-M): core c handles b = c//2,
    queries [h*4096, (h+1)*4096) with h = c%2, against the full gt[b].
  - Per core, compute the full 4096 x 8192 squared-distance matrix with
    the TensorEngine via an augmented K=5 matmul that yields d2 directly:
        lhsT rows = [qx, qy, qz, a2, 1]        (a2 = |q|^2)
        rhs  rows = [-2gx, -2gy, -2gz, 1, b2]  (b2 = |g|^2)
        out[m, n] = a2[m] + b2[n] - 2 q.g = d2[m, n]
    d2 is produced in PSUM [128 x 2048] tiles (4 banks), min-reduced over
    the free (n) axis on the VectorEngine, clamped at 0, sqrt'd on the
    ScalarEngine, and the per-query min distances [128, 32] DMA'd out.
  - Host gathers the 8 x [128, 32] min-distance arrays and takes the mean.
"""

import numpy as np

B, M, N, D = 4, 8192, 8192, 3
N_CORES = 8
MPC = (B * M) // N_CORES  # 4096 queries per core
M_TILES = MPC // 128  # 32
N_SUPER = 2048  # psum tile free size (4 banks)
N_GROUPS = N // N_SUPER  # 4
N_CHUNK = 512  # one matmul / one psum bank
LOSS_WEIGHT = 1.0

_CACHE: dict = {}


def _build_module():
    from contextlib import ExitStack

    import concourse.mybir as mybir
    import concourse.tile as tile
    from concourse import bacc

    nc = bacc.Bacc(
        "TRN2", target_bir_lowering=False, debug=False, num_devices=N_CORES
    )
    f32 = mybir.dt.float32
    qaug_d = nc.dram_tensor("qaug", [5, MPC], f32, kind="ExternalInput").ap()
    gaug_d = nc.dram_tensor("gaug", [5, N], f32, kind="ExternalInput").ap()
    mind_d = nc.dram_tensor("mind", [128, M_TILES], f32, kind="ExternalOutput").ap()

    with tile.TileContext(nc) as tc:
        with ExitStack() as ctx:
            inp = ctx.enter_context(tc.tile_pool(name="inp", bufs=1))
            psum = ctx.enter_context(tc.tile_pool(name="ps", bufs=2, space="PSUM"))
            small = ctx.enter_context(tc.tile_pool(name="sm", bufs=4))
            accp = ctx.enter_context(tc.tile_pool(name="acc", bufs=1))

            q_sb = inp.tile([5, MPC], f32)
            nc.sync.dma_start(q_sb[:], qaug_d[:])
            g_sb = inp.tile([5, N], f32)
            nc.sync.dma_start(g_sb[:], gaug_d[:])

            acc = accp.tile([128, M_TILES], f32)

            for mi in range(M_TILES):
                lhsT = q_sb[:, mi * 128 : (mi + 1) * 128]
                mins = small.tile([128, N_GROUPS], f32, tag="mins")
                for g in range(N_GROUPS):
                    pt = psum.tile([128, N_SUPER], f32, tag="pt")
                    for c in range(N_SUPER // N_CHUNK):
                        n0 = g * N_SUPER + c * N_CHUNK
                        nc.tensor.matmul(
                            pt[:, c * N_CHUNK : (c + 1) * N_CHUNK],
                            lhsT,
                            g_sb[:, n0 : n0 + N_CHUNK],
                            start=True,
                            stop=True,
                        )
                    nc.vector.tensor_reduce(
                        mins[:, g : g + 1],
                        pt[:],
                        axis=mybir.AxisListType.X,
                        op=mybir.AluOpType.min,
                    )
                dmin = small.tile([128, 1], f32, tag="dmin")
                nc.vector.tensor_reduce(
                    dmin[:],
                    mins[:],
                    axis=mybir.AxisListType.X,
                    op=mybir.AluOpType.min,
                )
                dclamp = small.tile([128, 1], f32, tag="dclamp")
                nc.vector.tensor_scalar_max(dclamp[:], dmin[:], 0.0)
                nc.scalar.activation(
                    acc[:, mi : mi + 1],
                    dclamp[:],
                    mybir.ActivationFunctionType.Sqrt,
                )

            nc.sync.dma_start(mind_d[:], acc[:])

    nc.compile()
    return nc


def _prep_in_maps(pred_colors: np.ndarray, gt_colors: np.ndarray):
    pred_colors = np.asarray(pred_colors, dtype=np.float32)
    gt_colors = np.asarray(gt_colors, dtype=np.float32)
    in_maps = []
    for c in range(N_CORES):
        b, h = divmod(c, N_CORES // B)
        q = pred_colors[b, h * MPC : (h + 1) * MPC]  # [MPC, 3]
        g = gt_colors[b]  # [N, 3]
        qaug = np.empty((5, MPC), dtype=np.float32)
        qaug[0:3] = q.T
        qaug[3] = (q * q).sum(axis=-1, dtype=np.float32)
        qaug[4] = 1.0
        gaug = np.empty((5, N), dtype=np.float32)
        gaug[0:3] = -2.0 * g.T
        gaug[3] = 1.0
        gaug[4] = (g * g).sum(axis=-1, dtype=np.float32)
        in_maps.append({"qaug": qaug, "gaug": gaug})
    return in_maps


def _get_module():
    if "nc" not in _CACHE:
        _CACHE["nc"] = _build_module()
    return _CACHE["nc"]


def kernel(pred_colors: np.ndarray, gt_colors: np.ndarray) -> np.ndarray:
    from concourse.bass_utils import run_bass_kernel_spmd

    nc = _get_module()
    in_maps = _prep_in_maps(pred_colors, gt_colors)
    res = run_bass_kernel_spmd(nc, in_maps, core_ids=list(range(N_CORES)))
    mins = np.stack([res.results[c]["mind"] for c in range(N_CORES)])
    out = np.mean(mins, dtype=np.float64) * LOSS_WEIGHT
    return np.asarray(out, dtype=np.float32)


# revision 22
# speedup vs baseline: 1.3958x; 1.3958x over previous
"""Trainium2 kernel for nn_ColorLoss (retrieval_knn).

Computes mean_{b,m} min_n ||pred[b,m] - gt[b,n]|| for B=4, M=N=8192, D=3.

Strategy (8 NeuronCores, SPMD):
  - Shard queries over (batch, half-of-M): core c handles b = c//2,
    queries [h*4096, (h+1)*4096) with h = c%2, against the full gt[b].
  - Per core: TensorEngine K=3 fp32 matmul produces ab = q.g in PSUM
    [128 x 2048] tiles (4 banks), exactly like XLA's einsum lowering.
  - A custom fused VectorEngine op computes, in a single 1x-rate pass,
        d2 = (a2[m] + b2[n]) + (-2)*ab[m,n]
    with the reference's rounding order ((a2+b2) rounded first, -2*ab
    exact), and min-reduces d2 over the free (n) axis via the accum path.
    Matching the rounding order matters: min-of-many amplifies fp32
    cancellation noise into a ~0.5% selection bias, so the kernel must
    reproduce the reference's noise statistics, not just its math.
  - Group mins are combined, clamped at 0, sqrt'd on the ScalarEngine and
    the per-query min distances [128, 32] are DMA'd out.
  - Host gathers the 8 x [128, 32] arrays and takes the mean.
"""

import numpy as np

B, M, N, D = 4, 8192, 8192, 3
N_CORES = 8
MPC = (B * M) // N_CORES  # 4096 queries per core
M_TILES = MPC // 128  # 32
N_SUPER = 2048  # psum tile free size (4 banks)
N_GROUPS = N // N_SUPER  # 4
N_CHUNK = 512  # one matmul / one psum bank
LOSS_WEIGHT = 1.0
BIG = 3.0e38

_CACHE: dict = {}


def _register_custom_op():
    """Runtime-register the fused (a2+b2)-2ab + min-reduce DVE op."""
    import concourse.dve_ops as dops
    from concourse.dve_spec import C0, C1, C2, Spec, Src0, Src1, lower, minn
    from concourse.dve_uop import DveOpSpec

    name = "COLORLOSS_D2MIN_ANT"
    for o in dops.OPS:
        if o.name == name:
            return o

    body = (Src1 + C0) + Src0 * C1

    def _ref(in0, in1, s0, s1, imm2):
        b = ((in1 + s0).astype(np.float32) + (in0 * s1).astype(np.float32)).astype(
            np.float32
        )
        acc = np.minimum(
            np.float32(imm2), b.reshape(b.shape[0], -1).min(axis=-1, keepdims=True)
        ).astype(np.float32)
        return b, acc

    spec = Spec(body=body, accum=minn, accum_init=C2, reference=_ref)
    row = dops._CUSTOM_DVE_ROW_BASE + len(dops.OPS)
    assert row < 0x20, "custom DVE row overflow"
    shas = {}
    for ver in ("v3", "v4"):
        s = DveOpSpec(name=name, opcode=row, uops=lower(spec, ver=ver), rd1_en=True)
        shas[ver] = s.sha(ver)
    op = dops.DveOp(name, spec, subdim=False, uops_sha=shas)
    dops.OPS.append(op)
    dops._SUB_OPCODE_FOR_NAME[name] = row
    return op


def _build_module(reps: int | None = None, ablation: str = "full"):
    """Build the SPMD module. reps=None is the production build; reps=R wraps
    the compute body in a For_i loop running it R times (timing builds).
    ablation: "full" | "pe_only" (skip DVE ops) | "dve_only" (skip matmuls) —
    timing probes only; results are garbage for != "full"."""
    from contextlib import ExitStack

    import concourse.mybir as mybir
    import concourse.tile as tile
    from concourse import bacc

    d2min_op = _register_custom_op()

    nc = bacc.Bacc(
        "TRN2", target_bir_lowering=False, debug=False, num_devices=N_CORES
    )
    f32 = mybir.dt.float32
    qT_d = nc.dram_tensor("qT", [3, MPC], f32, kind="ExternalInput").ap()
    gT_d = nc.dram_tensor("gT", [3, N], f32, kind="ExternalInput").ap()
    a2t_d = nc.dram_tensor("a2t", [128, M_TILES], f32, kind="ExternalInput").ap()
    b2r_d = nc.dram_tensor("b2r", [128, N], f32, kind="ExternalInput").ap()
    mind_d = nc.dram_tensor("mind", [128, M_TILES], f32, kind="ExternalOutput").ap()

    with tile.TileContext(nc) as tc:
        with ExitStack() as ctx:
            inp = ctx.enter_context(tc.tile_pool(name="inp", bufs=1))
            psum = ctx.enter_context(tc.tile_pool(name="ps", bufs=2, space="PSUM"))
            scr = ctx.enter_context(tc.tile_pool(name="scr", bufs=2))
            small = ctx.enter_context(tc.tile_pool(name="sm", bufs=4))
            accp = ctx.enter_context(tc.tile_pool(name="acc", bufs=1))

            # q/g replicated at partition bases {0,32,64,96} for 16-tile
            # row+col packed matmuls (K=3 uses 3 rows of each 32-row group).
            q_sb = inp.tile([128, MPC], f32)
            g_sb = inp.tile([128, N], f32)
            for i in range(4):
                nc.sync.dma_start(q_sb[32 * i : 32 * i + 3, :], qT_d[:])
                nc.sync.dma_start(g_sb[32 * i : 32 * i + 3, :], gT_d[:])
            a2_sb = inp.tile([128, M_TILES], f32)
            nc.sync.dma_start(a2_sb[:], a2t_d[:])
            b2_sb = inp.tile([128, N], f32)
            for g in range(N_GROUPS):  # chunked so later groups overlap compute
                sl = slice(g * N_SUPER, (g + 1) * N_SUPER)
                nc.sync.dma_start(b2_sb[:, sl], b2r_d[:, sl])

            acc = accp.tile([128, M_TILES], f32)

            def body():
                _emit_body(nc, tc, mybir, d2min_op, q_sb, g_sb, a2_sb, b2_sb, acc,
                           psum, scr, small, ablation)

            if reps is None:
                body()
            else:
                with tc.For_i(0, reps, 1):
                    body()

            nc.sync.dma_start(mind_d[:], acc[:])

    nc.compile()
    return nc


def _emit_body(nc, tc, mybir, d2min_op, q_sb, g_sb, a2_sb, b2_sb, acc, psum, scr,
               small, ablation="full"):
    f32 = mybir.dt.float32
    # All group mins land in one [128, M_TILES*N_GROUPS] tile; the combine /
    # clamp / sqrt run once at the end (small in-stream DVE ops are ~2us each).
    mins_all = small.tile([128, M_TILES * N_GROUPS], f32, tag="mins_all")
    for mi in range(M_TILES):
        for g in range(N_GROUPS):
            pt = psum.tile([128, N_SUPER], f32, tag="pt")
            if ablation != "dve_only":
                for i in range(4):  # row group: n-chunk of 512
                    n0 = g * N_SUPER + i * N_CHUNK
                    for j in range(4):  # col group: 32 queries
                        nc.tensor.matmul(
                            pt[32 * j : 32 * j + 32, i * N_CHUNK : (i + 1) * N_CHUNK],
                            q_sb[32 * i : 32 * i + 3, mi * 128 + 32 * j : mi * 128 + 32 * j + 32],
                            g_sb[32 * i : 32 * i + 3, n0 : n0 + N_CHUNK],
                            start=True,
                            stop=True,
                            tile_position=(32 * i, 32 * j),
                        )
            else:
                # touch the whole psum tile cheaply (N=16 x 4 banks) so the
                # custom op's read has a registered writer
                for i in range(4):
                    nc.tensor.matmul(
                        pt[:, i * N_CHUNK : i * N_CHUNK + 16],
                        q_sb[0:3, mi * 128 : mi * 128 + 128],
                        g_sb[0:3, 0:16],
                        start=True,
                        stop=True,
                    )
            if ablation != "pe_only":
                nc.vector._custom_dve(
                    d2min_op,
                    out=pt[:],  # in-place over in0: avoids an SBUF write stream
                    in0=pt[:],
                    in1=b2_sb[:, g * N_SUPER : (g + 1) * N_SUPER],
                    s0=a2_sb[:, mi : mi + 1],
                    s1=-2.0,
                    imm2=BIG,
                    accum_out=mins_all[:, mi * N_GROUPS + g : mi * N_GROUPS + g + 1],
                )
    if ablation == "pe_only":
        nc.gpsimd.memset(acc[:], 0.0)
        return
    dmin = small.tile([128, M_TILES], f32, tag="dmin")
    nc.vector.tensor_reduce(
        dmin[:],
        mins_all[:].rearrange("p (m g) -> p m g", g=N_GROUPS),
        axis=mybir.AxisListType.X,
        op=mybir.AluOpType.min,
    )
    dclamp = small.tile([128, M_TILES], f32, tag="dclamp")
    nc.scalar.activation(dclamp[:], dmin[:], mybir.ActivationFunctionType.Relu)
    nc.scalar.activation(acc[:], dclamp[:], mybir.ActivationFunctionType.Sqrt)


def _prep_in_maps(pred_colors: np.ndarray, gt_colors: np.ndarray):
    pred_colors = np.asarray(pred_colors, dtype=np.float32)
    gt_colors = np.asarray(gt_colors, dtype=np.float32)
    in_maps = []
    for c in range(N_CORES):
        b, h = divmod(c, N_CORES // B)
        q = pred_colors[b, h * MPC : (h + 1) * MPC]  # [MPC, 3]
        g = gt_colors[b]  # [N, 3]
        a2 = (q * q).sum(axis=-1, dtype=np.float32)
        b2 = (g * g).sum(axis=-1, dtype=np.float32)
        in_maps.append(
            {
                "qT": np.ascontiguousarray(q.T),
                "gT": np.ascontiguousarray(g.T),
                "a2t": np.ascontiguousarray(a2.reshape(M_TILES, 128).T),
                "b2r": np.ascontiguousarray(
                    np.broadcast_to(b2[None, :], (128, N))
                ),
            }
        )
    return in_maps


def _get_module(reps: int | None = None):
    key = ("nc", reps)
    if key not in _CACHE:
        _CACHE[key] = _build_module(reps)
    return _CACHE[key]


def kernel(pred_colors: np.ndarray, gt_colors: np.ndarray) -> np.ndarray:
    import time

    from concourse.bass_utils import run_bass_kernel_spmd

    nc = _get_module()
    in_maps = _prep_in_maps(pred_colors, gt_colors)
    last_err = None
    for attempt in range(3):  # first call after an unclean prior process can
        try:                  # hit a transient "device unrecoverable"; retry
            res = run_bass_kernel_spmd(nc, in_maps, core_ids=list(range(N_CORES)))
            break
        except Exception as e:  # noqa: BLE001
            last_err = e
            time.sleep(2.0)
            try:  # a fresh PJRT client clears terminal-side device state
                import jax

                jax.clear_backends()
            except Exception:  # noqa: BLE001
                pass
    else:
        raise last_err
    mins = np.stack([res.results[c]["mind"] for c in range(N_CORES)])
    out = np.mean(mins, dtype=np.float64) * LOSS_WEIGHT
    return np.asarray(out, dtype=np.float32)


# revision 33
# speedup vs baseline: 1.5018x; 1.0759x over previous
"""Trainium2 kernel for nn_ColorLoss (retrieval_knn).

Computes mean_{b,m} min_n ||pred[b,m] - gt[b,n]|| for B=4, M=N=8192, D=3.

Strategy (8 NeuronCores, SPMD):
  - Shard queries over (batch, half-of-M): core c handles b = c//2,
    queries [h*4096, (h+1)*4096) with h = c%2, against the full gt[b].
  - Per core: TensorEngine K=3 fp32 matmul produces ab = q.g in PSUM
    [128 x 2048] tiles (4 banks), exactly like XLA's einsum lowering.
  - A custom fused VectorEngine op computes, in a single 1x-rate pass,
        d2 = (a2[m] + b2[n]) + (-2)*ab[m,n]
    with the reference's rounding order ((a2+b2) rounded first, -2*ab
    exact), and min-reduces d2 over the free (n) axis via the accum path.
    Matching the rounding order matters: min-of-many amplifies fp32
    cancellation noise into a ~0.5% selection bias, so the kernel must
    reproduce the reference's noise statistics, not just its math.
  - Group mins are combined, clamped at 0, sqrt'd on the ScalarEngine and
    the per-query min distances [128, 32] are DMA'd out.
  - Host gathers the 8 x [128, 32] arrays and takes the mean.
"""

import numpy as np

B, M, N, D = 4, 8192, 8192, 3
N_CORES = 8
MPC = (B * M) // N_CORES  # 4096 queries per core
M_TILES = MPC // 128  # 32
N_SUPER = 2048  # psum tile free size (4 banks; x2 bufs = all of PSUM)
N_GROUPS = N // N_SUPER  # 4
N_CHUNK = 512  # one matmul / one psum bank
LOSS_WEIGHT = 1.0
BIG = 3.0e38

_CACHE: dict = {}


def _register_custom_op(swap: bool = False):
    """Runtime-register the fused (a2+b2)-2ab + min-reduce DVE op.

    swap=False: in0 = ab (PSUM), in1 = b2 (SBUF); body (Src1+C0) + Src0*C1.
    swap=True:  in0 = b2 (SBUF), in1 = ab (PSUM); body (Src0+C0) + Src1*C1.
    Same rounding order either way: (a2+b2) rounds, -2*ab exact, final rounds.
    """
    import concourse.dve_ops as dops
    from concourse.dve_spec import C0, C1, C2, Spec, Src0, Src1, lower, minn
    from concourse.dve_uop import DveOpSpec

    name = "COLORLOSS_D2MIN_SWAP_ANT" if swap else "COLORLOSS_D2MIN_ANT"
    for o in dops.OPS:
        if o.name == name:
            return o

    if swap:
        body = (Src0 + C0) + Src1 * C1

        def _ref(in0, in1, s0, s1, imm2):
            b = ((in0 + s0).astype(np.float32) + (in1 * s1).astype(np.float32)).astype(
                np.float32
            )
            acc = np.minimum(
                np.float32(imm2), b.reshape(b.shape[0], -1).min(axis=-1, keepdims=True)
            ).astype(np.float32)
            return b, acc
    else:
        body = (Src1 + C0) + Src0 * C1

        def _ref(in0, in1, s0, s1, imm2):
            b = ((in1 + s0).astype(np.float32) + (in0 * s1).astype(np.float32)).astype(
                np.float32
            )
            acc = np.minimum(
                np.float32(imm2), b.reshape(b.shape[0], -1).min(axis=-1, keepdims=True)
            ).astype(np.float32)
            return b, acc

    spec = Spec(body=body, accum=minn, accum_init=C2, reference=_ref)
    row = dops._CUSTOM_DVE_ROW_BASE + len(dops.OPS)
    assert row < 0x20, "custom DVE row overflow"
    shas = {}
    for ver in ("v3", "v4"):
        s = DveOpSpec(name=name, opcode=row, uops=lower(spec, ver=ver), rd1_en=True)
        shas[ver] = s.sha(ver)
    op = dops.DveOp(name, spec, subdim=False, uops_sha=shas)
    dops.OPS.append(op)
    dops._SUB_OPCODE_FOR_NAME[name] = row
    return op


SWAP_PORTS = False  # in0=ab (PSUM), in1=b2 (SBUF) in the custom op


def _build_module(reps: int | None = None, ablation: str = "full"):
    """Build the SPMD module. reps=None is the production build; reps=R wraps
    the compute body in a For_i loop running it R times (timing builds).
    ablation: "full" | "pe_only" (skip DVE ops) | "dve_only" (skip matmuls) —
    timing probes only; results are garbage for != "full"."""
    from contextlib import ExitStack

    import concourse.mybir as mybir
    import concourse.tile as tile
    from concourse import bacc

    d2min_op = _register_custom_op(swap=SWAP_PORTS)

    nc = bacc.Bacc(
        "TRN2", target_bir_lowering=False, debug=False, num_devices=N_CORES
    )
    f32 = mybir.dt.float32
    qT_d = nc.dram_tensor("qT", [3, MPC], f32, kind="ExternalInput").ap()
    gT_d = nc.dram_tensor("gT", [3, N], f32, kind="ExternalInput").ap()
    a2t_d = nc.dram_tensor("a2t", [128, M_TILES], f32, kind="ExternalInput").ap()
    b2r_d = nc.dram_tensor("b2r", [128, N], f32, kind="ExternalInput").ap()
    mind_d = nc.dram_tensor("mind", [128, M_TILES], f32, kind="ExternalOutput").ap()

    with tile.TileContext(nc) as tc:
        with ExitStack() as ctx:
            inp = ctx.enter_context(tc.tile_pool(name="inp", bufs=1))
            psum = ctx.enter_context(tc.tile_pool(name="ps", bufs=2, space="PSUM"))
            scr = ctx.enter_context(tc.tile_pool(name="scr", bufs=2))
            small = ctx.enter_context(tc.tile_pool(name="sm", bufs=4))
            accp = ctx.enter_context(tc.tile_pool(name="acc", bufs=1))

            # q/g replicated at partition bases {0,32,64,96}: each n-chunk's
            # K=3 matmul runs in its own 32-row group (4 concurrent tiles).
            q_sb = inp.tile([128, MPC], f32)
            g_sb = inp.tile([128, N], f32)
            for i in range(4):
                nc.sync.dma_start(q_sb[32 * i : 32 * i + 3, :], qT_d[:])
                nc.sync.dma_start(g_sb[32 * i : 32 * i + 3, :], gT_d[:])
            a2_sb = inp.tile([128, M_TILES], f32)
            nc.sync.dma_start(a2_sb[:], a2t_d[:])
            b2_sb = inp.tile([128, N], f32)
            for g in range(N_GROUPS):  # chunked so later groups overlap compute
                sl = slice(g * N_SUPER, (g + 1) * N_SUPER)
                nc.sync.dma_start(b2_sb[:, sl], b2r_d[:, sl])

            acc = accp.tile([128, M_TILES], f32)

            def body():
                _emit_body(nc, tc, mybir, d2min_op, q_sb, g_sb, a2_sb, b2_sb, acc,
                           psum, scr, small, ablation)

            if reps is None:
                body()
            else:
                with tc.For_i(0, reps, 1):
                    body()

            nc.sync.dma_start(mind_d[:], acc[:])

    nc.compile()
    return nc


def _emit_body(nc, tc, mybir, d2min_op, q_sb, g_sb, a2_sb, b2_sb, acc, psum, scr,
               small, ablation="full"):
    f32 = mybir.dt.float32
    # All group mins land in one [128, M_TILES*N_GROUPS] tile; the combine /
    # clamp / sqrt run once at the end (small in-stream DVE ops are ~2us each).
    mins_all = small.tile([128, M_TILES * N_GROUPS], f32, tag="mins_all")
    for mi in range(M_TILES):
        for g in range(N_GROUPS):
            pt_t = psum.tile([128, N_SUPER], f32, tag="pt")
            pt = pt_t[:]
            if ablation != "dve_only":
                for i in range(N_SUPER // N_CHUNK):  # n-chunks of 512, full M=128
                    n0 = g * N_SUPER + i * N_CHUNK
                    nc.tensor.matmul(
                        pt[:, i * N_CHUNK : (i + 1) * N_CHUNK],
                        q_sb[32 * (i % 4) : 32 * (i % 4) + 3, mi * 128 : (mi + 1) * 128],
                        g_sb[32 * (i % 4) : 32 * (i % 4) + 3, n0 : n0 + N_CHUNK],
                        start=True,
                        stop=True,
                        tile_position=(32 * (i % 4), 0),
                    )
            else:
                # touch the whole psum tile cheaply (N=16 x 4 banks) so the
                # custom op's read has a registered writer
                for i in range(4):
                    nc.tensor.matmul(
                        pt[:, i * N_CHUNK : i * N_CHUNK + 16],
                        q_sb[0:3, mi * 128 : mi * 128 + 128],
                        g_sb[0:3, 0:16],
                        start=True,
                        stop=True,
                    )
            if ablation != "pe_only":
                b2_slice = b2_sb[:, g * N_SUPER : (g + 1) * N_SUPER]
                srcs = (
                    dict(in0=b2_slice, in1=pt)
                    if SWAP_PORTS
                    else dict(in0=pt, in1=b2_slice)
                )
                nc.vector._custom_dve(
                    d2min_op,
                    out=pt,  # in-place over the psum half: no SBUF write
                    s0=a2_sb[:, mi : mi + 1],
                    s1=-2.0,
                    imm2=BIG,
                    accum_out=mins_all[:, mi * N_GROUPS + g : mi * N_GROUPS + g + 1],
                    **srcs,
                )
    if ablation == "pe_only":
        nc.gpsimd.memset(acc[:], 0.0)
        return
    dmin = small.tile([128, M_TILES], f32, tag="dmin")
    nc.vector.tensor_reduce(
        dmin[:],
        mins_all[:].rearrange("p (m g) -> p m g", g=N_GROUPS),
        axis=mybir.AxisListType.X,
        op=mybir.AluOpType.min,
    )
    dclamp = small.tile([128, M_TILES], f32, tag="dclamp")
    nc.scalar.activation(dclamp[:], dmin[:], mybir.ActivationFunctionType.Relu)
    nc.scalar.activation(acc[:], dclamp[:], mybir.ActivationFunctionType.Sqrt)


def _prep_in_maps(pred_colors: np.ndarray, gt_colors: np.ndarray):
    pred_colors = np.asarray(pred_colors, dtype=np.float32)
    gt_colors = np.asarray(gt_colors, dtype=np.float32)
    in_maps = []
    for c in range(N_CORES):
        b, h = divmod(c, N_CORES // B)
        q = pred_colors[b, h * MPC : (h + 1) * MPC]  # [MPC, 3]
        g = gt_colors[b]  # [N, 3]
        a2 = (q * q).sum(axis=-1, dtype=np.float32)
        b2 = (g * g).sum(axis=-1, dtype=np.float32)
        in_maps.append(
            {
                "qT": np.ascontiguousarray(q.T),
                "gT": np.ascontiguousarray(g.T),
                "a2t": np.ascontiguousarray(a2.reshape(M_TILES, 128).T),
                "b2r": np.ascontiguousarray(
                    np.broadcast_to(b2[None, :], (128, N))
                ),
            }
        )
    return in_maps


def _get_module(reps: int | None = None):
    key = ("nc", reps)
    if key not in _CACHE:
        _CACHE[key] = _build_module(reps)
    return _CACHE[key]


def kernel(pred_colors: np.ndarray, gt_colors: np.ndarray) -> np.ndarray:
    import time

    from concourse.bass_utils import run_bass_kernel_spmd

    nc = _get_module()
    in_maps = _prep_in_maps(pred_colors, gt_colors)
    last_err = None
    for attempt in range(3):  # first call after an unclean prior process can
        try:                  # hit a transient "device unrecoverable"; retry
            res = run_bass_kernel_spmd(nc, in_maps, core_ids=list(range(N_CORES)))
            break
        except Exception as e:  # noqa: BLE001
            last_err = e
            time.sleep(2.0)
            try:  # a fresh PJRT client clears terminal-side device state
                import jax

                jax.clear_backends()
            except Exception:  # noqa: BLE001
                pass
    else:
        raise last_err
    mins = np.stack([res.results[c]["mind"] for c in range(N_CORES)])
    out = np.mean(mins, dtype=np.float64) * LOSS_WEIGHT
    return np.asarray(out, dtype=np.float32)
